# revision 1
# baseline (speedup 1.0000x reference)
"""Trainium2 Bass kernel for a dense transformer encoder layer.

Shapes (from the problem): B=4, S=2048, D=512, H=8 heads (HD=64), FFN F=2048.

Sharding (8 NeuronCores, no collectives):
  core c handles batch b = c//2 and query-half  half = c%2 (1024 query tokens).
  K/V are computed for the full 2048-token sequence of the batch on both cores
  of a pair (duplicated K/V projection ~ +8% FLOPs, zero communication).

Device layout: all activations are kept TRANSPOSED ([d, token]) so every
matmul contraction has d on the PSUM partition axis and tokens are the moving
operand; LayerNorm statistics are computed with ones-vector matmuls on the PE
and applied via rank-1 broadcast matmuls. Softmax normalization uses an extra
ones-column appended to V (denominator rides along in the AV matmul).
The host transposes x / un-transposes the output (free in numpy).

Matmuls run as float32r (full fp32 storage, reduced-precision multiply) which
streams 1 column/cycle on TRN2 for moving dims >= 256 (4x faster than fp32).
"""

import functools
import numpy as np
from contextlib import ExitStack

import concourse.bass as bass
import concourse.tile as tile
import concourse.mybir as mybir
from concourse import bacc
from concourse.bass import ts
from concourse.vector_clock import ScopedClock

B, S, D, H, F = 4, 2048, 512, 8, 2048
HD = D // H           # 64
P = 128
DC = D // P           # 4  d chunks
FC = F // P           # 16 ffn chunks
SC = S // P           # 16 seq chunks
TOK = S // 2          # 1024 query tokens per core
NSL = TOK // 512      # 2 moving slices of 512
EPS = 1e-5
VW = HD + 1           # 65: V columns per head incl. ones column

f32 = mybir.dt.float32
f32r = mybir.dt.float32r
AF = mybir.ActivationFunctionType
ALU = mybir.AluOpType


class _TC(tile.TileContext):
    """TileContext whose tail drain splits sem waits one-per-drain: the
    walrus build in this container rejects >1 sync wait on an SP TPB_CTRL."""

    def _drain_and_barrier(self, tick_clock, wait_clock):
        nc = self.nc
        drain_inst = nc.sync.drain()
        wait_clock.add_sem_waits(
            drain_inst.ins, ScopedClock({None: tick_clock.global_clock})
        )
        si = drain_inst.ins.sync_info
        waits = list(si.on_wait) if si and si.on_wait else []
        MAXW = 1
        if len(waits) > MAXW:
            si.on_wait = waits[:MAXW]
            for i in range(MAXW, len(waits), MAXW):
                extra = nc.sync.drain()
                extra.ins.sync_info = mybir.SyncInfo(
                    on_wait=waits[i : i + MAXW], on_update=[]
                )
        nc.all_engine_barrier()
        popped = nc._tile_sem_poison_stack.pop()
        assert popped is self._sem_poison
        nc.clear_and_free_semaphores(list(self.sems.allocated().values()))
        nc.all_engine_barrier()


def _r(ap):
    return ap.bitcast(f32r)


def _bcast_ap(row_ap, nparts):
    """AP reading the single-partition row `row_ap` broadcast to nparts."""
    return bass.AP(
        tensor=row_ap.tensor,
        offset=row_ap.offset,
        ap=[[0, nparts]] + [list(d) for d in row_ap.ap[1:]],
    )


def _layernorm(tc, nc, src, gb_sb, dst, eps_sb, ones128, ones_row_d,
               stat_pool, work_pool, per_chunk_done=None):
    """dst[:,c,:] = LN(src) * g + beta, transposed layout [P, DC, TOK].

    Stats via ones-matmuls (sums over d on PE), application via rank-1
    broadcast matmuls: scale = g (x) rstd, shift = g (x) (-mu*rstd) + beta (x) 1.
    """
    with tc.tile_pool(name="ln_sum_ps", bufs=1, space="PSUM") as psS:
        sum_ps = psS.tile([1, TOK], f32, name="ln_sum")
        sq_ps = psS.tile([1, TOK], f32, name="ln_sqsum")
        for c in range(DC):
            sq = work_pool.tile([P, TOK], f32r, name="ln_sq", tag="ln_sq", bufs=2)
            nc.scalar.activation(sq, src[:, c, :], AF.Square)
            for sl in range(NSL):
                nc.tensor.matmul(
                    sum_ps[:, ts(sl, 512)], lhsT=_r(ones128),
                    rhs=_r(src[:, c, ts(sl, 512)]),
                    start=(c == 0), stop=(c == DC - 1), skip_group_check=True,
                )
                nc.tensor.matmul(
                    sq_ps[:, ts(sl, 512)], lhsT=_r(ones128),
                    rhs=_r(sq[:, ts(sl, 512)]),
                    start=(c == 0), stop=(c == DC - 1), skip_group_check=True,
                )
        # walrus requires all SBUF operands of one instruction to share the
        # same start partition -> every scratch vector lives at partition 0
        # in its own tiny tile (reused along the chain to save SBUF).
        tA = stat_pool.tile([1, TOK], f32, name="ln_tA", tag="ln_tA")
        tB = stat_pool.tile([1, TOK], f32, name="ln_tB", tag="ln_tB")
        tC = stat_pool.tile([1, TOK], f32r, name="ln_tC", tag="ln_tC")
        sh = stat_pool.tile([2, TOK], f32r, name="ln_sh", tag="ln_sh")
        shift_rhs = sh
        mu_neg = tA
        # run the serial stats chain in column halves so the two halves
        # pipeline (and the broadcast matmuls start after half 0 finishes)
        for hf in range(NSL):
            s5 = ts(hf, 512)
            nc.vector.tensor_scalar_mul(mu_neg[:, s5], sum_ps[:, s5], -1.0 / D)
            nc.vector.tensor_mul(tB[:, s5], mu_neg[:, s5], mu_neg[:, s5])
            # var = sq_sum/D - mu^2
            nc.vector.scalar_tensor_tensor(
                out=tC[:, s5], in0=sq_ps[:, s5], scalar=1.0 / D,
                in1=tB[:, s5], op0=ALU.mult, op1=ALU.subtract,
            )
    a_row = tC
    nc.sync.dma_start(out=sh[1:2, :], in_=ones_row_d[:])  # row 1 = ones
    for hf in range(NSL):
        s5 = ts(hf, 512)
        # std = sqrt(var + eps); a = 1/std ; b = -mu/std
        nc.scalar.activation(tB[:, s5], tC[:, s5], AF.Sqrt, bias=eps_sb,
                             scale=1.0)
        nc.vector.reciprocal(a_row[:, s5], tB[:, s5])
        nc.vector.tensor_mul(sh[0:1, s5], mu_neg[:, s5], a_row[:, s5])

    with tc.tile_pool(name="ln_bc_ps", bufs=2, space="PSUM") as psB:
        for c in range(DC):
            scale_ps = psB.tile([P, TOK], f32, name="ln_scale", tag="ln_scale")
            shift_ps = psB.tile([P, TOK], f32, name="ln_shift", tag="ln_shift")
            for sl in range(NSL):
                nc.tensor.matmul(
                    scale_ps[:, ts(sl, 512)], lhsT=_r(gb_sb[0:1, ts(c, P)]),
                    rhs=_r(a_row[:, ts(sl, 512)]), start=True, stop=True,
                )
                nc.tensor.matmul(
                    shift_ps[:, ts(sl, 512)], lhsT=_r(gb_sb[0:2, ts(c, P)]),
                    rhs=_r(shift_rhs[:, ts(sl, 512)]), start=True, stop=True,
                )
            t2 = work_pool.tile([P, TOK], f32, name="ln_t2", tag="ln_t2", bufs=2)
            nc.vector.tensor_tensor(t2, src[:, c, :], scale_ps, op=ALU.mult)
            nc.vector.tensor_tensor(dst[:, c, :], t2, shift_ps, op=ALU.add)
            if per_chunk_done is not None:
                per_chunk_done(c)


@functools.lru_cache(maxsize=1)
def _build_program():
    nc = bacc.Bacc()

    def dp(name, shape, out=False, dt=f32):
        return nc.declare_dram_parameter(name, list(shape), dt, isOutput=out)

    xbT_d = dp("xbT", [P, DC, S], dt=f32r)
    xqT_d = dp("xqT", [P, DC, TOK], dt=f32r)
    wqkv_d = dp("wqkv", [P, DC, 3 * D], dt=f32r)
    wo_d = dp("wo", [P, DC, D], dt=f32r)
    w1_d = dp("w1", [P, DC, F], dt=f32r)
    w2_d = dp("w2", [P, FC, D], dt=f32r)
    bqkvT_d = dp("bqkvT", [P, 12])
    boT_d = dp("boT", [P, DC])
    b1T_d = dp("b1T", [P, FC])
    b2T_d = dp("b2T", [P, DC])
    bvrow_d = dp("bvrow", [1, D])
    ones_col_d = dp("ones_col", [P, 1], dt=f32r)
    ones_row_d = dp("ones_row", [1, TOK], dt=f32r)
    vones_d = dp("vones", [P, SC, H, 1], dt=f32r)
    gb1_d = dp("gb1", [2, D], dt=f32r)
    gb2_d = dp("gb2", [2, D], dt=f32r)
    outT_d = dp("outT", [P, DC, TOK], out=True)

    with _TC(nc) as tc, ExitStack() as top:
        top.enter_context(
            nc.allow_low_precision(reason="float32r matmul pipeline by design")
        )
        persist = top.enter_context(tc.tile_pool(name="persist", bufs=1))
        bqkvT_sb = persist.tile([P, 12], f32)
        nc.sync.dma_start(out=bqkvT_sb, in_=bqkvT_d[:])
        boT_sb = persist.tile([P, DC], f32)
        nc.sync.dma_start(out=boT_sb, in_=boT_d[:])
        b1T_sb = persist.tile([P, FC], f32)
        nc.sync.dma_start(out=b1T_sb, in_=b1T_d[:])
        b2T_sb = persist.tile([P, DC], f32)
        nc.sync.dma_start(out=b2T_sb, in_=b2T_d[:])
        gb1_sb = persist.tile([2, D], f32r)
        nc.sync.dma_start(out=gb1_sb, in_=gb1_d[:])
        gb2_sb = persist.tile([2, D], f32r)
        nc.sync.dma_start(out=gb2_sb, in_=gb2_d[:])
        bvb_sb = persist.tile([P, D], f32)
        nc.gpsimd.dma_start(out=bvb_sb, in_=_bcast_ap(bvrow_d[:], P))
        ones128 = persist.tile([P, 1], f32r)
        nc.sync.dma_start(out=ones128, in_=ones_col_d[:])
        eps_sb = persist.tile([1, 1], f32)
        nc.vector.memset(eps_sb, EPS)
        ln1T_sb = persist.tile([P, DC, TOK], f32r)

        with ExitStack() as attn_scope:
            attnC = attn_scope.enter_context(tc.tile_pool(name="attnC", bufs=1))
            xqT_sb = attnC.tile([P, DC, TOK], f32r)
            nc.sync.dma_start(out=xqT_sb[:, 0, :], in_=xqT_d[:, 0, :])
            wo_sb = attnC.tile([P, DC, D], f32r)
            QT_sb = attnC.tile([P, DC, TOK], f32r)
            KT_sb = attnC.tile([P, DC, S], f32r)
            V_sb = attnC.tile([P, SC, H * VW], f32r)
            vcols = V_sb.rearrange("p k (h e) -> p k h e", e=VW)

            # ---------------- phase 1: QKV projections ----------------
            with tc.tile_pool(name="qkvB", bufs=1) as qkvB, \
                 tc.tile_pool(name="qkv_ps", bufs=2, space="PSUM") as ps1:
                wqkv_sb = qkvB.tile([P, DC, 3 * D], f32r)
                nc.sync.dma_start(out=wqkv_sb[:, 0, :], in_=wqkv_d[:, 0, :])
                for c in range(1, DC):
                    nc.sync.dma_start(out=xqT_sb[:, c, :], in_=xqT_d[:, c, :])
                    nc.sync.dma_start(out=wqkv_sb[:, c, :], in_=wqkv_d[:, c, :])
                xbT_sb = qkvB.tile([P, DC, S], f32r)
                for c in range(DC):
                    nc.sync.dma_start(out=xbT_sb[:, c, :], in_=xbT_d[:, c, :])
                # loads not needed until attention: V ones columns, Wo
                nc.sync.dma_start(out=vcols[:, :, :, HD:VW], in_=vones_d[:])
                nc.sync.dma_start(out=wo_sb, in_=wo_d[:])

                # Q^T [D, TOK]
                for m in range(DC):
                    q_ps = ps1.tile([P, TOK], f32, name="q_ps", tag="qk_ps")
                    for c in range(DC):
                        for sl in range(NSL):
                            nc.tensor.matmul(
                                q_ps[:, ts(sl, 512)],
                                lhsT=_r(wqkv_sb[:, c, ts(m, P)]),
                                rhs=_r(xqT_sb[:, c, ts(sl, 512)]),
                                start=(c == 0), stop=(c == DC - 1),
                            )
                    nc.vector.tensor_scalar_add(
                        QT_sb[:, m, :], q_ps, bqkvT_sb[:, m : m + 1]
                    )
                # K^T [D, S] (full sequence)
                for m in range(DC):
                    for half in range(2):
                        k_ps = ps1.tile([P, TOK], f32, name="k_ps", tag="qk_ps")
                        for c in range(DC):
                            for sl in range(NSL):
                                nc.tensor.matmul(
                                    k_ps[:, ts(sl, 512)],
                                    lhsT=_r(wqkv_sb[:, c, D + m * P : D + (m + 1) * P]),
                                    rhs=_r(xbT_sb[:, c, half * TOK + sl * 512 : half * TOK + (sl + 1) * 512]),
                                    start=(c == 0), stop=(c == DC - 1),
                                )
                        nc.vector.tensor_scalar_add(
                            KT_sb[:, m, half * TOK : (half + 1) * TOK],
                            k_ps, bqkvT_sb[:, 4 + m : 5 + m],
                        )
                # V natural [S, D] with per-head ones column (stride VW)
                bvb_h = bvb_sb.rearrange("p (h e) -> p h e", e=HD)
                for kc in range(SC):
                    v_ps = ps1.tile([P, D], f32, name="v_ps", tag="v_ps")
                    for c in range(DC):
                        nc.tensor.matmul(
                            v_ps,
                            lhsT=_r(xbT_sb[:, c, ts(kc, P)]),
                            rhs=_r(wqkv_sb[:, c, 2 * D : 3 * D]),
                            start=(c == 0), stop=(c == DC - 1),
                        )
                    nc.vector.tensor_tensor(
                        vcols[:, kc, :, 0:HD],
                        v_ps.rearrange("p (h e) -> p h e", e=HD),
                        bvb_h, op=ALU.add,
                    )

            # ---------------- phase 2: attention ----------------
            with ExitStack() as wscope:
                workD = wscope.enter_context(tc.tile_pool(name="workD", bufs=1))
                ctxT_sb = workD.tile([P, DC, TOK], f32r)
                res1_sb = workD.tile([P, DC, TOK], f32r)
                with tc.tile_pool(name="sc_ps", bufs=2, space="PSUM") as psSc, \
                     tc.tile_pool(name="ctx_ps", bufs=2, space="PSUM") as psCtx, \
                     tc.tile_pool(name="expP", bufs=3) as expP, \
                     tc.tile_pool(name="bcP", bufs=2) as bcP:
                    for h in range(H):
                        c4, r64 = h // 2, (h % 2) * HD
                        ctx_ps = psCtx.tile([VW, TOK], f32, name="ctx_ps", tag="ctx")
                        exps = []

                        def emit_av(kc, e, h=h, ctx_ps=ctx_ps):
                            for sl in range(NSL):
                                nc.tensor.matmul(
                                    ctx_ps[:, ts(sl, 512)],
                                    lhsT=_r(V_sb[:, kc, h * VW : (h + 1) * VW]),
                                    rhs=_r(e[:, ts(sl, 512)]),
                                    start=(kc == 0), stop=(kc == SC - 1),
                                    skip_group_check=True,
                                )

                        for kc in range(SC):
                            sc_ps = psSc.tile([P, TOK], f32, name="sc_ps", tag="sc")
                            for sl in range(NSL):
                                nc.tensor.matmul(
                                    sc_ps[:, ts(sl, 512)],
                                    lhsT=_r(KT_sb[r64 : r64 + HD, c4, ts(kc, P)]),
                                    rhs=_r(QT_sb[r64 : r64 + HD, c4, ts(sl, 512)]),
                                    start=True, stop=True, skip_group_check=True,
                                )
                            expT = expP.tile([P, TOK], f32r, name="expT", tag="expT")
                            nc.scalar.activation(expT, sc_ps, AF.Exp, scale=0.125)
                            exps.append(expT)
                            if kc >= 1:
                                emit_av(kc - 1, exps[kc - 1])
                        emit_av(SC - 1, exps[SC - 1])

                        recip = bcP.tile([1, TOK], f32, name="recip", tag="recip")
                        nc.vector.reciprocal(recip, ctx_ps[HD : HD + 1, :])
                        bc_sb = bcP.tile([P, TOK], f32, name="bc_sb", tag="bc")
                        nc.gpsimd.partition_broadcast(bc_sb, recip)
                        nc.vector.tensor_tensor(
                            ctxT_sb[r64 : r64 + HD, c4, :],
                            ctx_ps[0:HD, :], bc_sb[r64 : r64 + HD, :],
                            op=ALU.mult,
                        )

                # Wo projection + bias + residual
                with tc.tile_pool(name="wo_ps", bufs=2, space="PSUM") as psWo:
                    for m in range(DC):
                        wo_ps = psWo.tile([P, TOK], f32, name="wo_ps", tag="wo")
                        for c in range(DC):
                            for sl in range(NSL):
                                nc.tensor.matmul(
                                    wo_ps[:, ts(sl, 512)],
                                    lhsT=_r(wo_sb[:, c, ts(m, P)]),
                                    rhs=_r(ctxT_sb[:, c, ts(sl, 512)]),
                                    start=(c == 0), stop=(c == DC - 1),
                                )
                        nc.vector.scalar_tensor_tensor(
                            out=res1_sb[:, m, :], in0=wo_ps,
                            scalar=boT_sb[:, m : m + 1], in1=xqT_sb[:, m, :],
                            op0=ALU.add, op1=ALU.add,
                        )

                statP = wscope.enter_context(tc.tile_pool(name="statP", bufs=1))
                _layernorm(tc, nc, res1_sb, gb1_sb, ln1T_sb, eps_sb, ones128,
                           ones_row_d, statP, workD)

        # ---------------- phase 3: FFN ----------------
        with tc.tile_pool(name="ffnE", bufs=1) as ffnE, \
             tc.tile_pool(name="ffn_stat", bufs=1) as statF, \
             tc.tile_pool(name="ffn_work", bufs=2) as workF:
            w1_sb = ffnE.tile([P, DC, F], f32r)
            for c in range(DC):
                nc.sync.dma_start(out=w1_sb[:, c, :], in_=w1_d[:, c, :])
            w2_sb = ffnE.tile([P, FC, D], f32r)
            for c in range(FC):
                nc.sync.dma_start(out=w2_sb[:, c, :], in_=w2_d[:, c, :])
            hid_sb = ffnE.tile([P, FC, TOK], f32r)
            res2_sb = ffnE.tile([P, DC, TOK], f32r)
            # LN2 output reuses the first 4 chunks of hid (fully consumed by
            # then); saves 16KB/partition of SBUF
            out_sb = hid_sb[:, 0:DC, :]

            with tc.tile_pool(name="f1_ps", bufs=2, space="PSUM") as psF1:
                for m in range(FC):
                    h_ps = psF1.tile([P, TOK], f32, name="h_ps", tag="h")
                    for c in range(DC):
                        for sl in range(NSL):
                            nc.tensor.matmul(
                                h_ps[:, ts(sl, 512)],
                                lhsT=_r(w1_sb[:, c, ts(m, P)]),
                                rhs=_r(ln1T_sb[:, c, ts(sl, 512)]),
                                start=(c == 0), stop=(c == DC - 1),
                            )
                    nc.scalar.activation(
                        hid_sb[:, m, :], h_ps, AF.Relu, bias=b1T_sb[:, m : m + 1]
                    )
            with tc.tile_pool(name="f2_ps", bufs=2, space="PSUM") as psF2:
                for m in range(DC):
                    f_ps = psF2.tile([P, TOK], f32, name="f_ps", tag="f")
                    for c in range(FC):
                        for sl in range(NSL):
                            nc.tensor.matmul(
                                f_ps[:, ts(sl, 512)],
                                lhsT=_r(w2_sb[:, c, ts(m, P)]),
                                rhs=_r(hid_sb[:, c, ts(sl, 512)]),
                                start=(c == 0), stop=(c == FC - 1),
                            )
                    nc.vector.scalar_tensor_tensor(
                        out=res2_sb[:, m, :], in0=f_ps,
                        scalar=b2T_sb[:, m : m + 1], in1=ln1T_sb[:, m, :],
                        op0=ALU.add, op1=ALU.add,
                    )

            def _emit_out(c):
                nc.sync.dma_start(out=outT_d[:, c, :], in_=out_sb[:, c, :].bitcast(f32))

            _layernorm(tc, nc, res2_sb, gb2_sb, out_sb, eps_sb, ones128,
                       ones_row_d, statF, workF, per_chunk_done=_emit_out)

    if not nc.is_finalized():
        nc.finalize()
    return nc


def _prep_inputs(x, Wqkv, bqkv, Wo, bo, g1, beta1, W1, b1, W2, b2, g2, beta2):
    """Host-side sharding/layout prep -> list of 8 in_maps."""
    f = lambda a: np.ascontiguousarray(np.asarray(a, dtype=np.float32))

    def chunkT(w, nchunk):  # [n*128, cols] -> [128, n, cols]
        w = f(w)
        return np.ascontiguousarray(
            w.reshape(nchunk, P, w.shape[1]).transpose(1, 0, 2)
        )

    shared = {
        "wqkv": chunkT(Wqkv, DC),
        "wo": chunkT(Wo, DC),
        "w1": chunkT(W1, DC),
        "w2": chunkT(W2, FC),
        "bqkvT": f(np.asarray(bqkv).reshape(12, P).T),
        "boT": f(np.asarray(bo).reshape(DC, P).T),
        "b1T": f(np.asarray(b1).reshape(FC, P).T),
        "b2T": f(np.asarray(b2).reshape(DC, P).T),
        "bvrow": f(np.asarray(bqkv)[2 * D : 3 * D].reshape(1, D)),
        "ones_col": np.ones((P, 1), np.float32),
        "ones_row": np.ones((1, TOK), np.float32),
        "vones": np.ones((P, SC, H, 1), np.float32),
        "gb1": f(np.stack([np.asarray(g1), np.asarray(beta1)])),
        "gb2": f(np.stack([np.asarray(g2), np.asarray(beta2)])),
    }
    x = f(x)
    in_maps = []
    for c in range(8):
        b, half = c // 2, c % 2
        xbT = np.ascontiguousarray(
            x[b].T.reshape(DC, P, S).transpose(1, 0, 2)
        )
        xq = x[b, half * TOK : (half + 1) * TOK]
        xqT = np.ascontiguousarray(
            xq.T.reshape(DC, P, TOK).transpose(1, 0, 2)
        )
        in_maps.append(dict(shared, xbT=xbT, xqT=xqT))
    return in_maps


def kernel(**inputs):
    from concourse.bass_utils import run_bass_kernel_spmd

    nc = _build_program()
    in_maps = _prep_inputs(**inputs)
    res = run_bass_kernel_spmd(nc, in_maps, core_ids=list(range(8)))
    out = np.empty((B, S, D), dtype=np.float32)
    for c in range(8):
        b, half = c // 2, c % 2
        oT = res.results[c]["outT"]  # [P, DC, TOK]
        out[b, half * TOK : (half + 1) * TOK] = (
            oT.transpose(2, 1, 0).reshape(TOK, D)
        )
    return out



# revision 13
# speedup vs baseline: 1.3623x; 1.3623x over previous
"""Trainium2 Bass kernel for a dense transformer encoder layer.

Shapes: B=4, S=2048, D=512, H=8 heads (HD=64), FFN F=2048.

Sharding (8 NeuronCores, no collectives): core c handles batch b = c//2 and
query-half half = c%2 (1024 query tokens); K/V are computed for the full
2048-token sequence on both cores of a pair.

Precision/engine plan (the Activation engine's 128 softmax-exp tiles are the
~133us critical path; everything else hides under or around it):
  - QKV / Wo matmuls in bf16 (1 col/cycle, half the HBM bytes of f32).
  - Attention scores and attn@V in fp8 e4m3 with DoubleRow perf mode
    (0.5 cycles/row, 256-deep contraction): Q/K are stored in a "quad"
    layout ([32 partitions x 2 half-of-head k-tiles] per head) produced by
    host-permuting the Wq/Wk columns; exp() writes fp8 directly.
  - FFN f1/f2 in fp8 DoubleRow (weights quantized on host).
  - LayerNorm stats via ones-matmuls (f32r), applied with rank-1 broadcast
    matmuls; stats accumulate incrementally as residual chunks complete.
  - ReLU split across Act/DVE/Pool; V bias + squares + broadcasts on Pool.

Attention is a lag-1 pipeline: during head h's score/exp stretch the PE also
runs head h-1's attn@V and one woven aux-production group (V / K-quad1 /
Q-quad1) per score slot, all through a single 2-bank [128,512] PSUM ring, so
the Act engine never starves and PSUM stays within 8 banks
(scores 4 + ctx 2 + mix 2).
"""

import functools
import numpy as np
import ml_dtypes
from contextlib import ExitStack

import concourse.bass as bass
import concourse.tile as tile
import concourse.mybir as mybir
from concourse import bacc
from concourse.bass import ts
from concourse.vector_clock import ScopedClock

B, S, D, H, F = 4, 2048, 512, 8, 2048
HD = D // H           # 64
P = 128
DC = D // P           # 4  d chunks
FC = F // P           # 16 ffn chunks
SC = S // P           # 16 seq (key) chunks
TOK = S // 2          # 1024 query tokens per core
NSL = TOK // 512      # 2 moving slices of 512
EPS = 1e-5
VW = HD + 1           # 65: V columns per head incl. ones column
VWP = 80              # padded per-head V block (16B-aligned fp8 lhsT)

f32 = mybir.dt.float32
f32r = mybir.dt.float32r
bf16 = mybir.dt.bfloat16
f8 = mybir.dt.float8e4
AF = mybir.ActivationFunctionType
ALU = mybir.AluOpType
DR = mybir.MatmulPerfMode.DoubleRow


class _TC(tile.TileContext):
    """TileContext whose tail drain splits sem waits one-per-drain: the
    walrus build in this container rejects >1 sync wait on an SP TPB_CTRL."""

    def _drain_and_barrier(self, tick_clock, wait_clock):
        nc = self.nc
        drain_inst = nc.sync.drain()
        wait_clock.add_sem_waits(
            drain_inst.ins, ScopedClock({None: tick_clock.global_clock})
        )
        si = drain_inst.ins.sync_info
        waits = list(si.on_wait) if si and si.on_wait else []
        MAXW = 1
        if len(waits) > MAXW:
            si.on_wait = waits[:MAXW]
            for i in range(MAXW, len(waits), MAXW):
                extra = nc.sync.drain()
                extra.ins.sync_info = mybir.SyncInfo(
                    on_wait=waits[i : i + MAXW], on_update=[]
                )
        nc.all_engine_barrier()
        popped = nc._tile_sem_poison_stack.pop()
        assert popped is self._sem_poison
        nc.clear_and_free_semaphores(list(self.sems.allocated().values()))
        nc.all_engine_barrier()


def _r(ap):
    return ap.bitcast(f32r)


def _bcast_ap(row_ap, nparts):
    return bass.AP(
        tensor=row_ap.tensor,
        offset=row_ap.offset,
        ap=[[0, nparts]] + [list(d) for d in row_ap.ap[1:]],
    )


@functools.lru_cache(maxsize=1)
def _build_program():
    nc = bacc.Bacc()

    def dp(name, shape, out=False, dt=f32):
        return nc.declare_dram_parameter(name, list(shape), dt, isOutput=out)

    xbT_d = dp("xbT", [P, DC, S], dt=bf16)
    xqT_d = dp("xqT", [P, DC, TOK], dt=bf16)
    wqkv_d = dp("wqkv", [P, DC, 3 * D], dt=bf16)   # Q/K cols quad-permuted
    wo_d = dp("wo", [P, DC, D], dt=bf16)
    w18_d = dp("w18", [P, 2, 2, F], dt=f8)
    w28_d = dp("w28", [P, FC // 2, 2, D], dt=f8)
    bqkvT_d = dp("bqkvT", [P, 12])                 # Q/K cols quad-permuted
    boT_d = dp("boT", [P, DC])
    b1T_d = dp("b1T", [P, FC])
    b2T_d = dp("b2T", [P, DC])
    bvrow_d = dp("bvrow", [1, D])
    ones_col_d = dp("ones_col", [P, 1], dt=f32r)
    vones_d = dp("vones", [P, SC, H, 1], dt=f8)
    ones_row_d = dp("ones_row", [1, TOK], dt=f32r)
    gb1_d = dp("gb1", [2, D], dt=f32r)
    gb2_d = dp("gb2", [2, D], dt=f32r)
    outT_d = dp("outT", [P, DC, TOK], out=True)

    with _TC(nc) as tc, ExitStack() as top:
        top.enter_context(
            nc.allow_low_precision(reason="fp8/bf16 matmul pipeline by design")
        )
        persist = top.enter_context(tc.tile_pool(name="persist", bufs=1))
        bqkvT_sb = persist.tile([P, 12], f32)
        boT_sb = persist.tile([P, DC], f32)
        b1T_sb = persist.tile([P, FC], f32)
        b2T_sb = persist.tile([P, DC], f32)
        gb1_sb = persist.tile([2, D], f32r)
        gb2_sb = persist.tile([2, D], f32r)
        bvb_sb = persist.tile([P, D], f32)
        ones128 = persist.tile([P, 1], f32r)
        eps_sb = persist.tile([1, 1], f32)
        ln1T_sb = persist.tile([P, DC, TOK], f32r)
        ln18_sb = persist.tile([P, 2, 2, TOK], f8)
        sh_sb = persist.tile([2, TOK], f32r)       # row0: -mu*rstd, row1: ones

        nc.vector.memset(eps_sb, EPS)
        nc.sync.dma_start(out=ones128, in_=ones_col_d[:])
        nc.gpsimd.dma_start(out=bvb_sb, in_=_bcast_ap(bvrow_d[:], P))

        # ---------------- LN helper ----------------
        def make_ln(stat_pool, src, gb_sb, dst, work_pool, tag, fp8_cb=None,
                    done_cb=None):
            """Incremental layernorm over [P, DC, TOK] transposed layout."""

            def stat_cb(c, sum_ps, sq_ps):
                sq = work_pool.tile([P, TOK], f32r, name=f"sq{tag}",
                                    tag=f"sq{tag}", bufs=2)
                nc.gpsimd.tensor_mul(sq, src[:, c, :], src[:, c, :])
                for sl in range(NSL):
                    nc.tensor.matmul(
                        sum_ps[:, ts(sl, 512)], lhsT=ones128,
                        rhs=src[:, c, ts(sl, 512)],
                        start=(c == 0), stop=(c == DC - 1),
                        skip_group_check=True,
                    )
                    nc.tensor.matmul(
                        sq_ps[:, ts(sl, 512)], lhsT=ones128,
                        rhs=sq[:, ts(sl, 512)],
                        start=(c == 0), stop=(c == DC - 1),
                        skip_group_check=True,
                    )

            def serial(sum_ps, sq_ps):
                mu_neg = stat_pool.tile([1, TOK], f32, name=f"mu{tag}",
                                        tag=f"mu{tag}")
                tB = stat_pool.tile([1, TOK], f32, name=f"tB{tag}",
                                    tag=f"tB{tag}")
                var = stat_pool.tile([1, TOK], f32, name=f"var{tag}",
                                     tag=f"var{tag}")
                std = stat_pool.tile([1, TOK], f32, name=f"sd{tag}",
                                     tag=f"sd{tag}")
                rstd = stat_pool.tile([1, TOK], f32r, name=f"rs{tag}",
                                      tag=f"rs{tag}")
                for hf in range(NSL):
                    s5 = ts(hf, 512)
                    nc.vector.tensor_scalar_mul(mu_neg[:, s5], sum_ps[:, s5],
                                                -1.0 / D)
                    nc.vector.tensor_mul(tB[:, s5], mu_neg[:, s5],
                                         mu_neg[:, s5])
                    nc.vector.scalar_tensor_tensor(
                        out=var[:, s5], in0=sq_ps[:, s5], scalar=1.0 / D,
                        in1=tB[:, s5], op0=ALU.mult, op1=ALU.subtract,
                    )
                    nc.scalar.activation(std[:, s5], var[:, s5],
                                         AF.Sqrt, bias=eps_sb)
                    nc.vector.reciprocal(rstd[:, s5], std[:, s5])
                    nc.vector.tensor_mul(sh_sb[0:1, s5], mu_neg[:, s5],
                                         rstd[:, s5])
                return rstd

            def apply(rstd):
                with tc.tile_pool(name=f"bc{tag}", bufs=2, space="PSUM") as psB:
                    for c in range(DC):
                        scale_ps = psB.tile([P, TOK], f32, name=f"sc{tag}",
                                            tag=f"sc{tag}")
                        shift_ps = psB.tile([P, TOK], f32, name=f"sh{tag}",
                                            tag=f"sh{tag}")
                        for sl in range(NSL):
                            nc.tensor.matmul(
                                scale_ps[:, ts(sl, 512)],
                                lhsT=gb_sb[0:1, ts(c, P)],
                                rhs=rstd[:, ts(sl, 512)], start=True,
                                stop=True,
                            )
                            nc.tensor.matmul(
                                shift_ps[:, ts(sl, 512)],
                                lhsT=gb_sb[0:2, ts(c, P)],
                                rhs=sh_sb[:, ts(sl, 512)], start=True,
                                stop=True,
                            )
                        t2 = work_pool.tile([P, TOK], f32, name=f"t2{tag}",
                                            tag=f"t2{tag}", bufs=2)
                        nc.vector.tensor_tensor(t2, src[:, c, :], scale_ps,
                                                op=ALU.mult)
                        nc.vector.tensor_tensor(dst[:, c, :], t2, shift_ps,
                                                op=ALU.add)
                        if fp8_cb is not None:
                            fp8_cb(c)
                        if done_cb is not None:
                            done_cb(c)

            return stat_cb, serial, apply

        # ================ attention scope ================
        with ExitStack() as main:
            attnC = main.enter_context(tc.tile_pool(name="attnC", bufs=1))
            xbT_sb = attnC.tile([P, DC, S], bf16)
            xqT_sb = attnC.tile([P, DC, TOK], bf16)
            wqkv_sb = attnC.tile([P, DC, 3 * D], bf16)
            wo_sb = attnC.tile([P, DC, D], bf16)
            Q8 = attnC.tile([P, 2, 2, TOK], f8)    # [p, quad, dhalf, tok]
            K8 = attnC.tile([P, 2, 2, S], f8)      # [p, quad, dhalf, key]
            V8 = attnC.tile([P, SC, H * VWP], f8)  # [kpos, kc, h*80+e]
            vcols = V8.rearrange("p k (h e) -> p k h e", e=VWP)
            ctxT_sb = attnC.tile([P, DC, TOK], bf16)
            res1_sb = attnC.tile([P, DC, TOK], f32r)

            # DMA order tuned so first-exp deps land first.
            nc.sync.dma_start(out=bqkvT_sb, in_=bqkvT_d[:])
            nc.sync.dma_start(out=wqkv_sb[:, :, D : 2 * D],
                              in_=wqkv_d[:, :, D : 2 * D])
            nc.sync.dma_start(out=xbT_sb[:, :, 0:TOK], in_=xbT_d[:, :, 0:TOK])
            nc.sync.dma_start(out=wqkv_sb[:, :, 0:D], in_=wqkv_d[:, :, 0:D])
            nc.sync.dma_start(out=xqT_sb, in_=xqT_d[:])
            nc.sync.dma_start(out=xbT_sb[:, :, TOK:S], in_=xbT_d[:, :, TOK:S])
            nc.sync.dma_start(out=wqkv_sb[:, :, 2 * D : 3 * D],
                              in_=wqkv_d[:, :, 2 * D : 3 * D])
            nc.sync.dma_start(out=wo_sb, in_=wo_d[:])
            nc.sync.dma_start(out=boT_sb, in_=boT_d[:])
            nc.sync.dma_start(out=gb1_sb, in_=gb1_d[:])
            nc.sync.dma_start(out=sh_sb[1:2, :], in_=ones_row_d[:])
            nc.sync.dma_start(out=vcols[:, :, :, HD : HD + 1], in_=vones_d[:])

            attn_ps = main.enter_context(ExitStack())
            psSc = attn_ps.enter_context(
                tc.tile_pool(name="sc_ps", bufs=2, space="PSUM"))
            psCtx = attn_ps.enter_context(
                tc.tile_pool(name="ctx_ps", bufs=1, space="PSUM"))
            psMix = attn_ps.enter_context(
                tc.tile_pool(name="mix_ps", bufs=2, space="PSUM"))
            expP = attn_ps.enter_context(tc.tile_pool(name="expP", bufs=2))
            bcP = attn_ps.enter_context(tc.tile_pool(name="bcP", bufs=2))

            # --- aux production thunks (each ~0.85us of PE + a drain) ---
            def kq_thunk(proj, quad, dh, sl_abs):
                """One [P,512] piece of Q^T or K^T -> fp8 quad layout.

                proj 0=Q (tokens TOK wide), 1=K (keys S wide); sl_abs indexes
                512-wide column slices of the destination.
                """
                t = psMix.tile([P, 512], f32, name="mix", tag="mix")
                wbase = proj * D + (quad * 2 + dh) * P
                src = xqT_sb if proj == 0 else xbT_sb
                for c in range(DC):
                    nc.tensor.matmul(
                        t,
                        lhsT=wqkv_sb[:, c, wbase : wbase + P],
                        rhs=src[:, c, ts(sl_abs, 512)],
                        start=(c == 0), stop=(c == DC - 1),
                    )
                dst = Q8 if proj == 0 else K8
                nc.vector.tensor_scalar_add(
                    dst[:, quad, dh, ts(sl_abs, 512)], t,
                    bqkvT_sb[:, proj * 4 + quad * 2 + dh :
                             proj * 4 + quad * 2 + dh + 1],
                )

            bvb_h = bvb_sb.rearrange("p (h e) -> p h e", e=HD)

            def v_thunk(kc):
                t = psMix.tile([P, 512], f32, name="mix", tag="mix")
                for c in range(DC):
                    nc.tensor.matmul(
                        t,
                        lhsT=xbT_sb[:, c, ts(kc, P)],
                        rhs=wqkv_sb[:, c, 2 * D : 3 * D],
                        start=(c == 0), stop=(c == DC - 1),
                    )
                nc.vector.tensor_tensor(
                    vcols[:, kc, :, 0:HD],
                    t.rearrange("p (h e) -> p h e", e=HD),
                    bvb_h, op=ALU.add,
                )

            # --- phase A: K/Q quad 0 (first-exp deps first) ---
            for args in [(1, 0, 0, 0), (1, 0, 1, 0), (0, 0, 0, 0),
                         (0, 0, 1, 0), (0, 0, 0, 1), (0, 0, 1, 1),
                         (1, 0, 0, 1), (1, 0, 1, 1), (1, 0, 0, 2),
                         (1, 0, 1, 2), (1, 0, 0, 3), (1, 0, 1, 3)]:
                kq_thunk(*args)

            # --- phase B: lag-1 attention pipeline ---
            def emit_scores(h, kc, e8):
                quad, j = h // 4, h % 4
                sc_ps = psSc.tile([P, TOK], f32, name="sc_ps", tag="sc")
                for sl in range(NSL):
                    nc.tensor.matmul(
                        sc_ps[:, ts(sl, 512)],
                        lhsT=K8[32 * j : 32 * (j + 1), quad, :, ts(kc, P)],
                        rhs=Q8[32 * j : 32 * (j + 1), quad, :, ts(sl, 512)],
                        start=True, stop=True, perf_mode=DR,
                        skip_group_check=True, tile_position=(32 * j, 0),
                    )
                nc.scalar.activation(e8[:, kc, :], sc_ps, AF.Exp, scale=0.125)

            def emit_av(h, t, e8, ctx_ps):
                for sl in range(NSL):
                    nc.tensor.matmul(
                        ctx_ps[:, ts(sl, 512)],
                        lhsT=V8[:, 2 * t : 2 * t + 2, h * VWP : h * VWP + VW],
                        rhs=e8[:, 2 * t : 2 * t + 2, ts(sl, 512)],
                        start=(t == 0), stop=(t == SC // 2 - 1),
                        perf_mode=DR, skip_group_check=True,
                    )

            def drain_head(h, ctx_ps):
                c4, r64 = h // 2, (h % 2) * HD
                recip = bcP.tile([1, TOK], f32, name="recip", tag="recip")
                nc.vector.reciprocal(recip, ctx_ps[HD : HD + 1, :])
                bc_sb = bcP.tile([P, TOK], f32, name="bc_sb", tag="bc")
                nc.gpsimd.partition_broadcast(bc_sb, recip)
                nc.vector.tensor_tensor(
                    ctxT_sb[r64 : r64 + HD, c4, :],
                    ctx_ps[0:HD, :], bc_sb[r64 : r64 + HD, :], op=ALU.mult,
                )

            weave = {
                0: [functools.partial(v_thunk, kc) for kc in range(8)],
                1: [functools.partial(v_thunk, kc) for kc in range(8, 16)],
                2: [functools.partial(kq_thunk, 1, 1, dh, sl)
                    for sl in range(4) for dh in range(2)],
                3: [functools.partial(kq_thunk, 0, 1, dh, sl)
                    for sl in range(2) for dh in range(2)],
            }

            e8_tiles = {}

            def e8t(h):
                if h not in e8_tiles:
                    e8_tiles[h] = expP.tile([P, SC, TOK], f8, name="exp8",
                                            tag="exp8")
                return e8_tiles[h]

            def av_and_drain(h):
                ctx_ps = psCtx.tile([VW, TOK], f32, name="ctx_ps", tag="ctx")
                for t in range(SC // 2):
                    emit_av(h, t, e8t(h), ctx_ps)
                drain_head(h, ctx_ps)

            for h in range(H):
                items = list(weave.get(h, []))
                e8 = e8t(h)
                for kc in range(SC):
                    emit_scores(h, kc, e8)
                    if kc >= 1 and items:
                        items.pop(0)()
                while items:
                    items.pop(0)()
                if h > 0:
                    av_and_drain(h - 1)
            av_and_drain(H - 1)
            attn_ps.close()

            # ---- Wo + residual + LN1 (stats incremental) ----
            statP = main.enter_context(tc.tile_pool(name="statP", bufs=1))
            workP = main.enter_context(tc.tile_pool(name="workP", bufs=1))

            def ln1_fp8(c):
                nc.gpsimd.tensor_copy(ln18_sb[:, c // 2, c % 2, :],
                                      ln1T_sb[:, c, :])

            ln1_stat, ln1_serial, ln1_apply = make_ln(
                statP, res1_sb, gb1_sb, ln1T_sb, workP, "L1", fp8_cb=ln1_fp8)

            with tc.tile_pool(name="wo_ps", bufs=2, space="PSUM") as psWo, \
                 tc.tile_pool(name="st1_ps", bufs=1, space="PSUM") as psS1:
                sum1 = psS1.tile([1, TOK], f32, name="sum1")
                sq1 = psS1.tile([1, TOK], f32, name="sq1")
                for m in range(DC):
                    wo_ps = psWo.tile([P, TOK], f32, name="wo_ps", tag="wo")
                    for c in range(DC):
                        for sl in range(NSL):
                            nc.tensor.matmul(
                                wo_ps[:, ts(sl, 512)],
                                lhsT=wo_sb[:, c, ts(m, P)],
                                rhs=ctxT_sb[:, c, ts(sl, 512)],
                                start=(c == 0), stop=(c == DC - 1),
                            )
                    nc.vector.scalar_tensor_tensor(
                        out=res1_sb[:, m, :], in0=wo_ps,
                        scalar=boT_sb[:, m : m + 1], in1=xqT_sb[:, m, :],
                        op0=ALU.add, op1=ALU.add,
                    )
                    ln1_stat(m, sum1, sq1)
                rstd1 = ln1_serial(sum1, sq1)
            ln1_apply(rstd1)

        # ---------------- FFN + LN2 ----------------
        with tc.tile_pool(name="ffnE", bufs=1) as ffnE, \
             tc.tile_pool(name="ffn_stat", bufs=1) as statF, \
             tc.tile_pool(name="ffn_work", bufs=1) as workF:
            w18_sb = ffnE.tile([P, 2, 2, F], f8)
            w28_sb = ffnE.tile([P, FC // 2, 2, D], f8)
            nc.sync.dma_start(out=w18_sb, in_=w18_d[:])
            nc.sync.dma_start(out=w28_sb, in_=w28_d[:])
            nc.sync.dma_start(out=b1T_sb, in_=b1T_d[:])
            nc.sync.dma_start(out=b2T_sb, in_=b2T_d[:])
            nc.sync.dma_start(out=gb2_sb, in_=gb2_d[:])
            h8_sb = ffnE.tile([P, FC // 2, 2, TOK], f8)
            res2_sb = ffnE.tile([P, DC, TOK], f32r)
            out_sb = ffnE.tile([P, DC, TOK], f32)

            with tc.tile_pool(name="f1_ps", bufs=3, space="PSUM") as psF1:
                for m in range(FC):
                    h_ps = psF1.tile([P, TOK], f32, name="h_ps", tag="h")
                    for t in range(2):
                        for sl in range(NSL):
                            nc.tensor.matmul(
                                h_ps[:, ts(sl, 512)],
                                lhsT=w18_sb[:, t, :, ts(m, P)],
                                rhs=ln18_sb[:, t, :, ts(sl, 512)],
                                start=(t == 0), stop=(t == 1),
                                perf_mode=DR, skip_group_check=True,
                            )
                    dst = h8_sb[:, m // 2, m % 2, :]
                    if m % 2 == 0:
                        nc.scalar.activation(dst, h_ps, AF.Relu,
                                             bias=b1T_sb[:, m : m + 1])
                    else:
                        nc.vector.tensor_scalar(
                            out=dst, in0=h_ps, scalar1=b1T_sb[:, m : m + 1],
                            scalar2=0.0, op0=ALU.add, op1=ALU.max)

            def emit_out(c):
                nc.sync.dma_start(out=outT_d[:, c, :], in_=out_sb[:, c, :])

            ln2_stat, ln2_serial, ln2_apply = make_ln(
                statF, res2_sb, gb2_sb, out_sb, workF, "L2", done_cb=emit_out)

            with tc.tile_pool(name="f2_ps", bufs=2, space="PSUM") as psF2, \
                 tc.tile_pool(name="st2_ps", bufs=1, space="PSUM") as psS2:
                sum2 = psS2.tile([1, TOK], f32, name="sum2")
                sq2 = psS2.tile([1, TOK], f32, name="sq2")
                for m in range(DC):
                    f_ps = psF2.tile([P, TOK], f32, name="f_ps", tag="f")
                    for t in range(FC // 2):
                        for sl in range(NSL):
                            nc.tensor.matmul(
                                f_ps[:, ts(sl, 512)],
                                lhsT=w28_sb[:, t, :, ts(m, P)],
                                rhs=h8_sb[:, t, :, ts(sl, 512)],
                                start=(t == 0), stop=(t == FC // 2 - 1),
                                perf_mode=DR, skip_group_check=True,
                            )
                    nc.vector.scalar_tensor_tensor(
                        out=res2_sb[:, m, :], in0=f_ps,
                        scalar=b2T_sb[:, m : m + 1], in1=ln1T_sb[:, m, :],
                        op0=ALU.add, op1=ALU.add,
                    )
                    ln2_stat(m, sum2, sq2)
                rstd2 = ln2_serial(sum2, sq2)
            ln2_apply(rstd2)

    if not nc.is_finalized():
        nc.finalize()
    return nc


def _qk_perm():
    """perm[m, p] -> original column (within one D block) for Q/K chunk m,
    where chunk m = (quad, dhalf) and partition p = 32*j + idx for head
    j = p//32 of the quad."""
    perm = np.zeros((DC, P), np.int64)
    p = np.arange(P)
    for q in range(2):
        for dh in range(2):
            perm[q * 2 + dh] = (q * 4 + p // 32) * HD + dh * 32 + (p % 32)
    return perm


def _prep_inputs(x, Wqkv, bqkv, Wo, bo, g1, beta1, W1, b1, W2, b2, g2, beta2):
    f = lambda a: np.ascontiguousarray(np.asarray(a, dtype=np.float32))
    to_bf = lambda a: np.ascontiguousarray(
        np.asarray(a, np.float32).astype(ml_dtypes.bfloat16))
    to_f8 = lambda a: np.ascontiguousarray(
        np.asarray(a, np.float32).astype(ml_dtypes.float8_e4m3fn))

    def chunkT(w, nchunk):  # [n*128, cols] -> [128, n, cols]
        w = np.asarray(w, np.float32)
        return np.ascontiguousarray(
            w.reshape(nchunk, P, w.shape[1]).transpose(1, 0, 2)
        )

    perm = _qk_perm()
    colperm = np.concatenate(
        [perm.reshape(-1), D + perm.reshape(-1), 2 * D + np.arange(D)]
    )
    Wqkv_p = np.asarray(Wqkv, np.float32)[:, colperm]
    bqkv_p = np.asarray(bqkv, np.float32)[colperm]

    W1f = np.asarray(W1, np.float32)
    W2f = np.asarray(W2, np.float32)
    # [p, t, i, cols] = W[(2t+i)*128+p, cols]
    w18 = to_f8(W1f.reshape(2, 2, P, F).transpose(2, 0, 1, 3))
    w28 = to_f8(W2f.reshape(FC // 2, 2, P, D).transpose(2, 0, 1, 3))

    shared = {
        "wqkv": to_bf(chunkT(Wqkv_p, DC)),
        "wo": to_bf(chunkT(np.asarray(Wo, np.float32), DC)),
        "w18": w18,
        "w28": w28,
        "bqkvT": f(bqkv_p.reshape(12, P).T),
        "boT": f(np.asarray(bo).reshape(DC, P).T),
        "b1T": f(np.asarray(b1).reshape(FC, P).T),
        "b2T": f(np.asarray(b2).reshape(DC, P).T),
        "bvrow": f(np.asarray(bqkv)[2 * D : 3 * D].reshape(1, D)),
        "ones_row": np.ones((1, TOK), np.float32),
        "ones_col": np.ones((P, 1), np.float32),
        "vones": np.ones((P, SC, H, 1), np.float32).astype(
            ml_dtypes.float8_e4m3fn),
        "gb1": f(np.stack([np.asarray(g1), np.asarray(beta1)])),
        "gb2": f(np.stack([np.asarray(g2), np.asarray(beta2)])),
    }
    x = np.asarray(x, np.float32)
    in_maps = []
    for c in range(8):
        b, half = c // 2, c % 2
        xbT = to_bf(x[b].T.reshape(DC, P, S).transpose(1, 0, 2))
        xq = x[b, half * TOK : (half + 1) * TOK]
        xqT = to_bf(xq.T.reshape(DC, P, TOK).transpose(1, 0, 2))
        in_maps.append(dict(shared, xbT=xbT, xqT=xqT))
    return in_maps


def kernel(**inputs):
    from concourse.bass_utils import run_bass_kernel_spmd

    nc = _build_program()
    in_maps = _prep_inputs(**inputs)
    res = run_bass_kernel_spmd(nc, in_maps, core_ids=list(range(8)))
    out = np.empty((B, S, D), dtype=np.float32)
    for c in range(8):
        b, half = c // 2, c % 2
        oT = res.results[c]["outT"]  # [P, DC, TOK]
        out[b, half * TOK : (half + 1) * TOK] = (
            oT.transpose(2, 1, 0).reshape(TOK, D)
        )
    return out
